# revision 4
# baseline (speedup 1.0000x reference)
"""CrossAttention (3x 3D-conv projections + channel attention + residual)
on 8 Trainium2 NeuronCores, data-parallel over batch (2 batches/core).

Reference computation (B=16, C=1024, D=H=W=8, N=D*H*W=512):
  q = conv3d(x, wq, bq); k = conv3d(y, wk, bk); v = conv3d(y, wv, bv)
  scores[b,n,m] = sum_c q[b,c,n] k[b,c,m]; attn = softmax(scores, -1)
  out[b,c,n] = sum_m attn[b,n,m] v[b,c,m];  return out + x

Per-core kernel (all matmuls fp32r: full PE rate at free-dim 512,
~1.5e-4 component relative error):
  - convs: for each kernel tap t (27) and input-channel chunk ic (8),
    a [K=128 x M=128] weight tile against a [K=128 x N=512] shifted slab
    of the zero-padded activation (nested strided AP), accumulated in
    PSUM over all 216 (t, ic) steps; both batches + 4 output chunks
    share each weight-tile DMA (8 PSUM banks live).
  - q/k/vT are spilled to DRAM scratch after each conv (SBUF can't hold
    the padded activations and all projection outputs at once) and
    reloaded per batch for the attention phase.
  - v is transposed on the TensorEngine (128x128 identity-matmul blocks)
    to give vT[m, c] for the second attention matmul.
  - bq/bk are added during PSUM evacuation; bv is folded into the
    residual (softmax rows sum to 1, so attn @ (v + bv) = attn @ v + bv).
"""
import sys

sys.path.insert(0, '/opt/trn_rl_repo')

import numpy as np

from concourse import bacc, mybir, masks
from concourse.tile import TileContext
from concourse.bass_utils import run_bass_kernel_spmd

F32 = mybir.dt.float32
F32R = mybir.dt.float32r

B, C, N = 16, 1024, 512
NCORES = 8
BPC = B // NCORES          # batches per core
ICH = C // 128             # input/output channel chunks of 128
PAD = 1000                 # 10*10*10 padded volume

_CACHED_NC = None
LAST_RESULTS = None


def _taps():
    for t in range(27):
        yield t, t // 9, (t // 3) % 3, t % 3


def _build():
    nc = bacc.Bacc("TRN2", target_bir_lowering=False, debug=False)

    xp = nc.dram_tensor("xp", [BPC, C, PAD], F32R, kind="ExternalInput")
    yp = nc.dram_tensor("yp", [BPC, C, PAD], F32R, kind="ExternalInput")
    xres = nc.dram_tensor("xres", [BPC, C, N], F32, kind="ExternalInput")
    wqp = nc.dram_tensor("wqp", [27, C, C], F32R, kind="ExternalInput")
    wkp = nc.dram_tensor("wkp", [27, C, C], F32R, kind="ExternalInput")
    wvp = nc.dram_tensor("wvp", [27, C, C], F32R, kind="ExternalInput")
    bqp = nc.dram_tensor("bqp", [128, ICH], F32, kind="ExternalInput")
    bkp = nc.dram_tensor("bkp", [128, ICH], F32, kind="ExternalInput")
    out = nc.dram_tensor("out", [BPC, C, N], F32, kind="ExternalOutput")

    # DRAM scratch for projection outputs between conv and attention
    q_dr = nc.dram_tensor("q_dr", [BPC, 128, ICH, N], F32R)
    k_dr = nc.dram_tensor("k_dr", [BPC, 128, ICH, N], F32R)
    vT_dr = nc.dram_tensor("vT_dr", [BPC, 128, 4, C], F32R)

    with TileContext(nc) as tc:
        with tc.tile_pool(name="const", bufs=1) as cpool, \
             tc.tile_pool(name="psum", bufs=1, space="PSUM") as psp:

            ident = cpool.tile([128, 128], F32, tag="ident")
            masks.make_identity(nc, ident[:])
            bq_t = cpool.tile([128, ICH], F32, tag="bq_t")
            nc.sync.dma_start(bq_t[:], bqp[:])
            bk_t = cpool.tile([128, ICH], F32, tag="bk_t")
            nc.sync.dma_start(bk_t[:], bkp[:])

            def psum_tile(i):
                return psp.tile([128, 512], F32, tag=f"ps{i}", name=f"ps{i}")

            with tc.tile_pool(name="acts", bufs=1) as apool, \
                 tc.tile_pool(name="wts", bufs=8) as wpool, \
                 tc.tile_pool(name="ev", bufs=4) as evpool:

                # padded activations: [128, ic 8, 10, 10, 10]
                def load_pad(src, name):
                    tiles = []
                    for b in range(BPC):
                        t = apool.tile([128, ICH, 10, 10, 10], F32R,
                                       tag=f"{name}{b}", name=f"{name}{b}")
                        nc.sync.dma_start(
                            t[:].rearrange("p i a b c -> p i (a b c)"),
                            src[b].rearrange("(i p) n -> p i n", p=128))
                        tiles.append(t)
                    return tiles

                xpad = load_pad(xp, "xpad")
                ypad = load_pad(yp, "ypad")

                def conv(pads, w_dram, bias_t, dst_dr, transposed):
                    """Accumulate the 27x8-step conv in PSUM, then evacuate
                    to DRAM scratch (with bias, or TensorE-transposed)."""
                    for oh in range(2):
                        pq = [psum_tile(i) for i in range(8)]
                        for t, kd, kh, kw in _taps():
                            for ic in range(ICH):
                                wt = wpool.tile([128, 512], F32R, tag="wt",
                                                name="wt")
                                nc.sync.dma_start(
                                    wt[:],
                                    w_dram[t, ic * 128:(ic + 1) * 128,
                                           oh * 512:(oh + 1) * 512])
                                first = (t == 0 and ic == 0)
                                last = (t == 26 and ic == ICH - 1)
                                for ol in range(4):
                                    lhsT = wt[:, ol * 128:(ol + 1) * 128]
                                    for b in range(BPC):
                                        slab = pads[b][:, ic, kd:kd + 8,
                                                       kh:kh + 8, kw:kw + 8]
                                        nc.tensor.matmul(
                                            pq[ol * BPC + b][:], lhsT, slab,
                                            start=first, stop=last)
                        for ol in range(4):
                            oc = oh * 4 + ol
                            for b in range(BPC):
                                g = ol * BPC + b
                                if not transposed:
                                    stage = evpool.tile([128, 512], F32R,
                                                        tag="stage",
                                                        name="stage")
                                    nc.vector.tensor_scalar_add(
                                        stage[:], pq[g][:],
                                        bias_t[:, oc:oc + 1])
                                    nc.sync.dma_start(dst_dr[b][:, oc, :],
                                                      stage[:])
                                else:
                                    vtmp = evpool.tile([128, 512], F32,
                                                       tag="vtmp", bufs=2,
                                                       name="vtmp")
                                    nc.vector.tensor_copy(vtmp[:], pq[g][:])
                                    ptv = psum_tile(g)
                                    for mc in range(4):
                                        nc.tensor.transpose(
                                            ptv[:, mc * 128:(mc + 1) * 128],
                                            vtmp[:, mc * 128:(mc + 1) * 128],
                                            ident[:])
                                    stage2 = evpool.tile([128, 4, 128], F32R,
                                                         tag="stage2", bufs=2,
                                                         name="stage2")
                                    nc.vector.tensor_copy(
                                        stage2[:],
                                        ptv[:].rearrange("p (m c) -> p m c",
                                                         m=4))
                                    nc.sync.dma_start(
                                        vT_dr[b][:, :,
                                                 oc * 128:(oc + 1) * 128],
                                        stage2[:])

                conv(xpad, wqp, bq_t, q_dr, False)
                conv(ypad, wkp, bk_t, k_dr, False)
                conv(ypad, wvp, None, vT_dr, True)

            # ---------------- attention + residual ----------------
            with tc.tile_pool(name="attn", bufs=1) as dpool, \
                 tc.tile_pool(name="ot", bufs=4) as opool:
                for b in range(BPC):
                    q_t = dpool.tile([128, ICH, N], F32R, tag=f"q_t{b}",
                                     name=f"q_t{b}")
                    nc.sync.dma_start(q_t[:], q_dr[b])
                    k_t = dpool.tile([128, ICH, N], F32R, tag=f"k_t{b}",
                                     name=f"k_t{b}")
                    nc.sync.dma_start(k_t[:], k_dr[b])
                    vT_t = dpool.tile([128, 4, C], F32R, tag=f"vT_t{b}",
                                      name=f"vT_t{b}")
                    nc.sync.dma_start(vT_t[:], vT_dr[b])
                    xr = dpool.tile([128, ICH, N], F32, tag=f"xr{b}",
                                    name=f"xr{b}")
                    nc.sync.dma_start(
                        xr[:], xres[b].rearrange("(i p) n -> p i n", p=128))

                    # scores + softmax, 4 token chunks of 128 rows
                    stats = dpool.tile([128, 3, 4], F32, tag="stats",
                                       name="stats")
                    attn_n = dpool.tile([128, 4, N], F32, tag="attn_n",
                                        name="attn_n")
                    for g in range(4):
                        ps = psum_tile(g)
                        for oc in range(ICH):
                            nc.tensor.matmul(
                                ps[:],
                                q_t[:, oc, g * 128:(g + 1) * 128],
                                k_t[:, oc, :],
                                start=(oc == 0), stop=(oc == ICH - 1))
                        negmax = stats[:, 0, g:g + 1]
                        esum = stats[:, 1, g:g + 1]
                        rinv = stats[:, 2, g:g + 1]
                        nc.vector.reduce_max(negmax, ps[:],
                                             axis=mybir.AxisListType.X,
                                             negate=True)
                        nc.scalar.activation(attn_n[:, g, :], ps[:],
                                             mybir.ActivationFunctionType.Exp,
                                             bias=negmax, accum_out=esum)
                        nc.vector.reciprocal(rinv, esum)
                        nc.vector.tensor_scalar_mul(attn_n[:, g, :],
                                                    attn_n[:, g, :], rinv)
                    # transpose attn -> attnT [m-part, mc, n]
                    attnT = dpool.tile([128, 4, N], F32R, tag="attnT",
                                       name="attnT")
                    for mc in range(4):
                        pt = psum_tile(4 + mc)
                        for g in range(4):
                            nc.tensor.transpose(
                                pt[:, g * 128:(g + 1) * 128],
                                attn_n[:, g, mc * 128:(mc + 1) * 128],
                                ident[:])
                        nc.vector.tensor_copy(attnT[:, mc, :], pt[:])
                    # out = vT.T @ attnT + (x + bv)
                    for oc in range(ICH):
                        po = psum_tile(oc)
                        for mc in range(4):
                            nc.tensor.matmul(
                                po[:],
                                vT_t[:, mc, oc * 128:(oc + 1) * 128],
                                attnT[:, mc, :],
                                start=(mc == 0), stop=(mc == 3))
                        ot = opool.tile([128, N], F32, tag="ot", name="ot")
                        nc.vector.tensor_add(ot[:], po[:], xr[:, oc, :])
                        nc.sync.dma_start(
                            out[b, oc * 128:(oc + 1) * 128, :], ot[:])
    nc.compile()
    return nc


def _prep_weight(w):
    # [O, I, kd, kh, kw] -> [t, i, o] contiguous
    return np.ascontiguousarray(
        w.transpose(2, 3, 4, 1, 0).reshape(27, C, C)).astype(np.float32)


def kernel(x, y, wq, bq, wk, bk, wv, bv):
    global _CACHED_NC, LAST_RESULTS
    x = np.asarray(x, np.float32)
    y = np.asarray(y, np.float32)

    xf = x.reshape(B, C, 8, 8, 8)
    yf = y.reshape(B, C, 8, 8, 8)
    xpad = np.zeros((B, C, 10, 10, 10), np.float32)
    xpad[:, :, 1:9, 1:9, 1:9] = xf
    ypad = np.zeros((B, C, 10, 10, 10), np.float32)
    ypad[:, :, 1:9, 1:9, 1:9] = yf
    xpad = xpad.reshape(B, C, PAD)
    ypad = ypad.reshape(B, C, PAD)
    xres = x.reshape(B, C, N) + np.asarray(bv, np.float32)[None, :, None]

    wqp = _prep_weight(np.asarray(wq, np.float32))
    wkp = _prep_weight(np.asarray(wk, np.float32))
    wvp = _prep_weight(np.asarray(wv, np.float32))
    bqp = np.ascontiguousarray(
        np.asarray(bq, np.float32).reshape(ICH, 128).T)
    bkp = np.ascontiguousarray(
        np.asarray(bk, np.float32).reshape(ICH, 128).T)

    if _CACHED_NC is None:
        _CACHED_NC = _build()

    in_maps = []
    for i in range(NCORES):
        s = slice(i * BPC, (i + 1) * BPC)
        in_maps.append({
            "xp": xpad[s], "yp": ypad[s], "xres": xres[s],
            "wqp": wqp, "wkp": wkp, "wvp": wvp,
            "bqp": bqp, "bkp": bkp,
        })

    res = run_bass_kernel_spmd(_CACHED_NC, in_maps, list(range(NCORES)))
    LAST_RESULTS = res
    full = np.concatenate([res.results[i]["out"] for i in range(NCORES)],
                          axis=0)
    return full.reshape(B, C, 8, 8, 8)
